# revision 35
# baseline (speedup 1.0000x reference)
"""Trainium2 Bass kernel for a 2-layer ConvGRU (L=512, T=96, C=H=150, K=5).

Sharding: spatial axis L split over 8 NeuronCores (64 owned positions each).
Each core computes a 128-wide region (owned + 32 halo per side). Halo
validity decays 2 positions/step (kernel-5 SAME conv); it is refreshed every
16 steps by a ReduceScatter halo exchange (per-core one-hot masks route each
core's boundary strips to its neighbours' slots, keeping the SPMD program
uniform). Layer-0 exchanges sit at t=16k, layer-1 at t=16k+8.

v2 restructure vs the original baseline (959us):
- h state kept in bf16 end-to-end; PE transposes cost 1 cyc/row.
- gate math rewritten as h' = s*n + (prev - s*prev) with s = sigmoid(-zpre)
  computed directly on ACT (scale=-1), so only 2 DVE ops follow the tanh.
- replica copies moved from DVE to the otherwise idle GpSimd engine.
- x-side matmuls of step t+1 are emitted between this slot's h matmuls and
  the gate-dependent PE transposes, so the PE never idles during gate math
  (keeps the HAM activity monitor at the full 2.4 GHz clock).
- exchange rs DMAs moved to the SP queue so the Pool replica stream never
  blocks on the collective.

All input reshaping (channel-major transposes, tap-shifted im2col packing,
bias/valid rows, routing masks) is done host-side in numpy inside kernel().
"""

import os
import sys
import types

import numpy as np
import ml_dtypes

BF16 = ml_dtypes.bfloat16

if "/opt/trn_rl_repo" not in sys.path:
    sys.path.insert(0, "/opt/trn_rl_repo")


def _install_ntff_hook():
    # antenv.axon_hooks is absent from this image; recreate the registry and
    # register the ctypes NTFF hook so trace=True yields exec_time_ns.
    try:
        import antenv
        if "antenv.axon_hooks" in sys.modules:
            return True
        mod = types.ModuleType("antenv.axon_hooks")
        _hook = [None]
        mod.set_axon_ntff_profile_hook = lambda h: _hook.__setitem__(0, h)
        mod.get_axon_ntff_profile_hook = lambda: _hook[0]
        sys.modules["antenv.axon_hooks"] = mod
        antenv.axon_hooks = mod
        from trn_agent_boot.trn_boot import _ntff_profile_via_ctypes
        mod.set_axon_ntff_profile_hook(
            _ntff_profile_via_ctypes("/opt/axon/libaxon_pjrt.so"))
        return True
    except Exception:
        return False


L, C, H, KW = 512, 150, 150, 5
G3 = 3 * H            # 450
NCORES = 8
OWN = 64              # owned positions per core
SW = 134              # stored width per timestep (128 region + pads)
W0 = 25               # rolling slots for layer-0 output trajectory
LAG = 4               # emission lag of layer 1 behind layer 0
CL = 22               # channel remainder (150 - 128)
AR = 118              # 4 tap blocks at partitions {0,32,64,96}, 22 rows each
A5 = 119              # A-block rows: AR gap-filled + ones/bias row at 118

# tap-4 channel -> spare row inside the 118-row aligned block (layer-0 xs
# host packing only): gaps 22..31, 54..63, 86..87; ones/bias row at 88.
_GAP_PACK = [(22 + c, c) for c in range(10)] + \
    [(54 + c - 10, c) for c in range(10, 20)] + \
    [(86 + c - 20, c) for c in range(20, 22)]
_ONES_ROW = 88        # ones/bias row for the host-packed layer-0 x A-block

LAST_EXEC_NS = None


def build(T):
    import concourse.bacc as bacc
    import concourse.mybir as mybir
    import concourse.tile as tile

    f32 = mybir.dt.float32
    bf16 = mybir.dt.bfloat16
    AF = mybir.ActivationFunctionType
    nc = bacc.Bacc("TRN2", target_bir_lowering=False, debug=False,
                   num_devices=NCORES)

    d_xsa = nc.dram_tensor("xsT_a", [128, T * SW], bf16, kind="ExternalInput")
    d_xsba = nc.dram_tensor("xsT_bA", [AR, T * SW], bf16, kind="ExternalInput")
    # weights are packed host-side into few wide tensors: DMA issue cost is
    # per partition-row descriptor, so fewer/wider transfers start faster
    wnames = [("wmm0", [128, 2 * KW * G3]), ("wmm1", [128, 2 * KW * G3]),
              ("wA0", [A5, 2 * G3]), ("wA1", [A5, 2 * G3]),
              ("wB", [CL, 3 * G3])]
    d_w = {nm: nc.dram_tensor(nm, sh, bf16, kind="ExternalInput")
           for nm, sh in wnames}
    d_maska = nc.dram_tensor("mask_a", [128, 512], bf16, kind="ExternalInput")
    d_maskb = nc.dram_tensor("mask_b", [CL, 512], bf16, kind="ExternalInput")
    d_valid = nc.dram_tensor("valid128", [1, 128], bf16, kind="ExternalInput")
    d_valid25 = nc.dram_tensor("valid25", [1, W0 * 128], bf16,
                               kind="ExternalInput")
    d_validp = nc.dram_tensor("validp", [128, 1], f32, kind="ExternalInput")
    d_identb = nc.dram_tensor("identb", [128, 128], bf16, kind="ExternalInput")
    d_out = nc.dram_tensor("out", [T, OWN, H], bf16, kind="ExternalOutput")

    with tile.TileContext(nc) as tc:
        with tc.tile_pool(name="persist", bufs=1) as pp, \
             tc.tile_pool(name="work", bufs=2) as wp, \
             tc.tile_pool(name="hnew", bufs=3) as hp, \
             tc.tile_pool(name="psP", bufs=2, space="PSUM") as psP, \
             tc.tile_pool(name="psQ", bufs=2, space="PSUM") as psQ, \
             tc.tile_pool(name="dram", bufs=2, space="DRAM") as dp:

            # ---- persistent SBUF tiles ----
            xsa = pp.tile([128, T * SW], bf16, tag="xsa", name="xsa")
            xsba = pp.tile([AR, T * SW], bf16, tag="xsba", name="xsba")
            w_sb = {nm: pp.tile(sh, bf16, tag=nm, name=nm) for nm, sh in wnames}
            maska = pp.tile([128, 512], bf16, tag="maska", name="maska")
            maskb = pp.tile([CL, 512], bf16, tag="maskb", name="maskb")
            valid = pp.tile([1, 128], bf16, tag="valid", name="valid")
            validp = pp.tile([128, 1], f32, tag="validp", name="validp")
            identb = pp.tile([128, 128], bf16, tag="identb", name="identb")
            hz0 = pp.tile([128, 152], bf16, tag="hz0", name="hz0")
            ys0a = pp.tile([128, W0 * SW], bf16, tag="ys0a", name="ys0a")
            ys0b = pp.tile([CL, W0 * SW], bf16, tag="ys0b", name="ys0b")
            ys5 = pp.tile([A5, W0 * 128], bf16, tag="ys5", name="ys5")
            h1a = pp.tile([128, SW], bf16, tag="h1a", name="h1a")
            h1b = pp.tile([CL, SW], bf16, tag="h1b", name="h1b")
            h51 = pp.tile([A5, 128], bf16, tag="h51", name="h51")

            # ---- init: loads + zero state ----
            # DMA issue cost is ~29ns/descriptor (one per partition row), so
            # order matters more than size: step-1 needs land first, big
            # trailing chunks later; xsa/weights ride the sync queue while
            # xsba rides the scalar HWDGE queue in parallel.
            tcuts = [0, 4, 16, 48, 96]
            ci = 0
            for a, b in zip(tcuts, tcuts[1:]):
                a, b = a * SW, min(b, T) * SW
                if b > a:
                    nc.sync.dma_start(xsa[:, a:b], d_xsa[:, a:b])
                    nc.scalar.dma_start(xsba[:, a:b], d_xsba[:, a:b])
                if ci == 0:
                    nc.sync.dma_start(w_sb["wmm0"][:], d_w["wmm0"][:])
                    nc.sync.dma_start(w_sb["wA0"][:], d_w["wA0"][:])
                    nc.scalar.dma_start(w_sb["wB"][:], d_w["wB"][:])
                    nc.scalar.dma_start(validp[:], d_validp[:])
                    nc.scalar.dma_start(identb[:], d_identb[:])
                    nc.scalar.dma_start(ys5[A5 - 1:A5, :], d_valid25[0:1, :])
                elif ci == 1:
                    nc.sync.dma_start(w_sb["wmm1"][:], d_w["wmm1"][:])
                    nc.scalar.dma_start(w_sb["wA1"][:], d_w["wA1"][:])
                ci += 1
            nc.scalar.dma_start(maska[:], d_maska[:])
            nc.scalar.dma_start(maskb[:], d_maskb[:])
            nc.scalar.dma_start(valid[:], d_valid[:])
            wrm_in = dp.tile([8, 64], f32, tag="wrm_in", name="wrm_in")
            wrm_out = dp.tile([1, 64], f32, tag="wrm_out", name="wrm_out")
            wrm_sb = wp.tile([8, 64], f32, tag="wrm_sb", name="wrm_sb")
            nc.vector.memset(wrm_sb[:], 0.0)
            nc.sync.dma_start(wrm_in[:], wrm_sb[:])
            nc.gpsimd.collective_compute(
                "ReduceScatter", mybir.AluOpType.add,
                replica_groups=[list(range(NCORES))],
                ins=[wrm_in[:].opt()], outs=[wrm_out[:].opt()])
            nc.vector.memset(ys0a[:], 0.0)
            nc.vector.memset(ys0b[:], 0.0)
            nc.vector.memset(hz0[:], 0.0)
            nc.gpsimd.memset(ys5[0:A5 - 1, :], 0.0)
            nc.gpsimd.memset(h1a[:], 0.0)
            nc.gpsimd.memset(h1b[:], 0.0)
            nc.gpsimd.memset(h51[:], 0.0)

            def xphase(l, t):
                """x-side gate pre-activations of step t into a fresh PSUM
                bank (450 wide, bias via the A-block ones row)."""
                P = psP.tile([128, G3], f32, tag=f"P{l}", name=f"P{l}")
                if l == 0:
                    xa, xbase = xsa, (t - 1) * SW
                    xA, xAb, xAr = xsba, (t - 1) * SW, AR
                else:
                    xa, xbase = ys0a, (t % W0) * SW
                    xA, xAb, xAr = ys5, (t % W0) * 128, A5
                for k in range(KW):
                    nc.tensor.matmul(
                        P[:, 0:G3], xa[:, xbase + k: xbase + k + 128],
                        w_sb[f"wmm{l}"][:, k * G3:(k + 1) * G3],
                        start=(k == 0), stop=False)
                if l == 1:
                    nc.tensor.matmul(
                        P[:, 0:G3], ys0b[0:CL, (t % W0) * SW + 4:
                                         (t % W0) * SW + 132],
                        w_sb["wB"][0:CL, 2 * G3:3 * G3],
                        start=False, stop=False)
                nc.tensor.matmul(P[:, 0:G3], xA[0:xAr, xAb: xAb + 128],
                                 w_sb[f"wA{l}"][0:xAr, 0:G3],
                                 start=False, stop=False)
                return P

            def hphase(l, t, P):
                """h-side conv accumulation: [r|z] into P, n into Q."""
                Q = psQ.tile([128, 512], f32, tag=f"Q{l}", name=f"Q{l}")
                if l == 0:
                    ha, hbase = ys0a, ((t - 1) % W0) * SW
                    hA, hAb = ys5, ((t - 1) % W0) * 128
                    hb, hBb = ys0b, ((t - 1) % W0) * SW + 4
                else:
                    ha, hbase = h1a, 0
                    hA, hAb = h51, 0
                    hb, hBb = h1b, 4
                hm = KW * G3
                bb = G3 * l
                for k in range(KW):
                    nc.tensor.matmul(
                        P[:, 0:2 * H], ha[:, hbase + k: hbase + k + 128],
                        w_sb[f"wmm{l}"][:, hm + k * G3: hm + k * G3 + 2 * H],
                        start=False, stop=False)
                    nc.tensor.matmul(
                        Q[:, 0:H], ha[:, hbase + k: hbase + k + 128],
                        w_sb[f"wmm{l}"][:, hm + k * G3 + 2 * H:
                                        hm + (k + 1) * G3],
                        start=(k == 0), stop=False)
                nc.tensor.matmul(P[:, 0:2 * H], hb[0:CL, hBb: hBb + 128],
                                 w_sb["wB"][0:CL, bb: bb + 2 * H],
                                 start=False, stop=False)
                nc.tensor.matmul(Q[:, 0:H], hb[0:CL, hBb: hBb + 128],
                                 w_sb["wB"][0:CL, bb + 2 * H: bb + G3],
                                 start=False, stop=False)
                nc.tensor.matmul(P[:, 0:2 * H], hA[0:A5, hAb: hAb + 128],
                                 w_sb[f"wA{l}"][0:A5, G3: G3 + 2 * H],
                                 start=False, stop=True)
                nc.tensor.matmul(Q[:, 0:H], hA[0:A5, hAb: hAb + 128],
                                 w_sb[f"wA{l}"][0:A5, G3 + 2 * H: 2 * G3],
                                 start=False, stop=True)
                return Q

            def gates(l, t, prev, P, Q):
                """h' = s*n + (prev - s*prev), s = sigmoid(-zpre) = 1-z."""
                r = wp.tile([128, 152], f32, tag=f"r{l}", name=f"r{l}")
                nc.scalar.activation(r[:, 0:H], P[:, 0:H], AF.Sigmoid)
                s = wp.tile([128, 152], f32, tag=f"s{l}", name=f"s{l}")
                nc.scalar.activation(s[:, 0:H], P[:, H:2 * H], AF.Sigmoid,
                                     scale=-1.0)
                ta = wp.tile([128, 152], f32, tag=f"ta{l}", name=f"ta{l}")
                nc.vector.tensor_mul(ta[:, 0:H], Q[:, 0:H], r[:, 0:H])
                tb = wp.tile([128, 152], f32, tag=f"tb{l}", name=f"tb{l}")
                nc.vector.tensor_add(tb[:, 0:H], ta[:, 0:H], P[:, 2 * H:G3])
                # scale by the valid-position indicator: pins n to exactly 0
                # outside the global [0,L) range (reference SAME padding)
                tn = wp.tile([128, 152], f32, tag=f"tn{l}", name=f"tn{l}")
                nc.scalar.activation(tn[:, 0:H], tb[:, 0:H], AF.Tanh,
                                     scale=validp[:, 0:1])
                # off-critical-path while tanh runs (GpSimd is otherwise
                # idle): w_ = prev - s*prev
                tsp = wp.tile([128, 152], f32, tag=f"tsp{l}", name=f"tsp{l}")
                nc.gpsimd.tensor_mul(tsp[:, 0:H], s[:, 0:H], prev[:, 0:H])
                w_ = wp.tile([128, 152], f32, tag=f"w{l}", name=f"w{l}")
                nc.gpsimd.tensor_sub(w_[:, 0:H], prev[:, 0:H], tsp[:, 0:H])
                e = wp.tile([128, 152], f32, tag=f"e{l}", name=f"e{l}")
                nc.vector.tensor_mul(e[:, 0:H], s[:, 0:H], tn[:, 0:H])
                hn = hp.tile([128, 152], bf16, tag=f"hn{l}", name=f"hn{l}")
                nc.vector.tensor_add(hn[:, 0:H], e[:, 0:H], w_[:, 0:H])
                return hn

            def transposes(l, t, hn):
                """h_new back to channel-major via 2 PE transposes (bf16)."""
                TT = psQ.tile([128, 256], bf16, tag=f"Q{l}", name=f"TT{l}")
                nc.tensor.matmul(TT[:, 0:128], hn[:, 0:128], identb[:],
                                 is_transpose=True, start=True, stop=True)
                nc.tensor.matmul(TT[0:CL, 128:256], hn[:, 128:H], identb[:],
                                 is_transpose=True, start=True, stop=True)
                return TT

            def copies(l, t, TT, hn):
                if l == 0:
                    st = (t % W0) * SW
                    nc.vector.tensor_copy(ys0a[:, st + 2: st + 130],
                                          TT[:, 0:128])
                    nc.scalar.copy(ys0b[0:CL, st + 2: st + 130],
                                   TT[0:CL, 128:256])
                else:
                    nc.vector.tensor_copy(h1a[:, 2:130], TT[:, 0:128])
                    nc.scalar.copy(h1b[0:CL, 2:130], TT[0:CL, 128:256])
                    nc.sync.dma_start(d_out[t - 1, :, :], hn[32:96, 0:H])

            def replicas_tt(l, t, TT):
                """Rebuild the 32-aligned tap blocks 0..3 straight from the
                transpose PSUM (no wait on the ysb copy). Strip col j lives
                at TT col 126+j; the out-of-strip edge columns only feed
                halo-edge output positions, which the decay budget already
                writes off. k=3 is clipped one column to avoid reading
                stale PSUM beyond the transpose (its last A column keeps
                old slot data — also halo-edge-only)."""
                if l == 0:
                    s5 = (t % W0) * 128
                    A_ = ys5
                else:
                    s5 = 0
                    A_ = h51
                for k in range(4):
                    w = 127 if k == 3 else 128
                    eng = nc.vector if k < 2 else nc.scalar
                    src = TT[0:CL, 126 + k: 126 + k + w]
                    if eng is nc.vector:
                        eng.tensor_copy(A_[k * 32: k * 32 + CL, s5: s5 + w], src)
                    else:
                        eng.copy(A_[k * 32: k * 32 + CL, s5: s5 + w], src)

            def replicas_sb(l, t):
                """Post-exchange rebuild of the tap blocks from the patched
                channel-remainder strip (proper zero pads at the edges)."""
                if l == 0:
                    st = (t % W0) * SW
                    s5 = (t % W0) * 128
                    A_, B_ = ys5, ys0b
                else:
                    st, s5 = 0, 0
                    A_, B_ = h51, h1b
                for k in range(4):
                    src = B_[0:CL, st + k: st + k + 128]
                    if k < 2:
                        nc.vector.tensor_copy(
                            A_[k * 32: k * 32 + CL, s5: s5 + 128], src)
                    else:
                        nc.scalar.copy(
                            A_[k * 32: k * 32 + CL, s5: s5 + 128], src)

            def exchange_comm(l, t):
                """ReduceScatter halo refresh of layer l's state at step t.
                Mask muls on DVE, rs DMAs on SP; only the CC trigger sits on
                the Pool stream (emitted after this slot's replicas)."""
                if l == 0:
                    A, B, base = ys0a, ys0b, (t % W0) * SW
                else:
                    A, B, base = h1a, h1b, 0
                rsa = wp.tile([128, 512], bf16, tag="rsa", name="rsa")
                rsb = wp.tile([CL, 512], bf16, tag="rsb", name="rsb")
                # q in [0,32): receivers' left halo <- my owned last 32
                # q in [32,64): receivers' right halo <- my owned first 32
                for dq, sc in ((0, 66), (32, 34)):
                    nc.vector.tensor_mul(
                        rsa[:].rearrange("c (s q) -> c s q", s=8)[:, :, dq:dq + 32],
                        A[:, base + sc: base + sc + 32]
                        .unsqueeze(1).broadcast_to([128, 8, 32]),
                        maska[:].rearrange("c (s q) -> c s q", s=8)[:, :, dq:dq + 32])
                    nc.vector.tensor_mul(
                        rsb[0:CL].rearrange("c (s q) -> c s q", s=8)[:, :, dq:dq + 32],
                        B[0:CL, base + sc: base + sc + 32]
                        .unsqueeze(1).broadcast_to([CL, 8, 32]),
                        maskb[:].rearrange("c (s q) -> c s q", s=8)[:, :, dq:dq + 32])
                rs_in = dp.tile([8, C, 64], bf16, tag="rs_in", name="rs_in")
                rs_out = dp.tile([C, 64], bf16, tag="rs_out", name="rs_out")
                nc.sync.dma_start(rs_in[:, 0:128, :].transpose([1, 0, 2]),
                                  rsa[:].rearrange("c (s q) -> c s q", s=8))
                nc.sync.dma_start(rs_in[:, 128:C, :].transpose([1, 0, 2]),
                                  rsb[0:CL].rearrange("c (s q) -> c s q", s=8))

                def go():
                    nc.gpsimd.collective_compute(
                        "ReduceScatter", mybir.AluOpType.add,
                        replica_groups=[list(range(NCORES))],
                        ins=[rs_in[:].opt()], outs=[rs_out[:].opt()])
                    nc.sync.dma_start(A[:, base + 2: base + 34],
                                      rs_out[0:128, 0:32])
                    nc.sync.dma_start(A[:, base + 98: base + 130],
                                      rs_out[0:128, 32:64])
                    nc.sync.dma_start(B[0:CL, base + 2: base + 34],
                                      rs_out[128:C, 0:32])
                    nc.sync.dma_start(B[0:CL, base + 98: base + 130],
                                      rs_out[128:C, 32:64])
                return go

            def exchange_patch(l, t, prev):
                """Deferred: re-materialise patched halo rows of the
                position-major h_t copy via PE transposes (emitted next slot
                so in-order engine streams don't stall on the collective)."""
                if l == 0:
                    A, B, base = ys0a, ys0b, (t % W0) * SW
                else:
                    A, B, base = h1a, h1b, 0
                TX = psQ.tile([32, 512], bf16, tag=f"Q{l}", name=f"TX{l}")
                nc.tensor.matmul(TX[0:32, 0:128], A[:, base + 2: base + 34],
                                 identb[:], is_transpose=True,
                                 start=True, stop=True)
                nc.tensor.matmul(TX[0:32, 128:256], A[:, base + 98: base + 130],
                                 identb[:], is_transpose=True,
                                 start=True, stop=True)
                nc.tensor.matmul(TX[0:32, 256:256 + CL],
                                 B[0:CL, base + 2: base + 34],
                                 identb[0:CL, 0:CL], is_transpose=True,
                                 start=True, stop=True)
                nc.tensor.matmul(TX[0:32, 288:288 + CL],
                                 B[0:CL, base + 98: base + 130],
                                 identb[0:CL, 0:CL], is_transpose=True,
                                 start=True, stop=True)
                nc.vector.tensor_copy(prev[0:32, 0:128], TX[0:32, 0:128])
                nc.vector.tensor_copy(prev[96:128, 0:128], TX[0:32, 128:256])
                nc.vector.tensor_copy(prev[0:32, 128:H], TX[0:32, 256:256 + CL])
                nc.vector.tensor_copy(prev[96:128, 128:H],
                                      TX[0:32, 288:288 + CL])

            # ---- main pipelined emission ----
            prev0, prev1 = hz0, hz0
            hn0 = hn1 = None
            pend0, pend1 = [], []
            xpend0, xpend1 = {}, {}
            xpend0[1] = xphase(0, 1)
            swap = False

            for t0 in range(1, T + LAG + 1):
                t1 = t0 - LAG
                do0 = t0 <= T
                do1 = 1 <= t1 <= T
                ex0 = do0 and t0 % 16 == 0
                ex1 = do1 and t1 % 16 == 8 and t1 < T

                def head0():
                    nonlocal hn0
                    for f in pend0:
                        f()
                    pend0.clear()
                    if not do0:
                        return
                    P0 = xpend0.pop(t0)
                    Q0 = hphase(0, t0, P0)
                    hn0 = gates(0, t0, prev0, P0, Q0)

                def head1():
                    nonlocal hn1
                    for f in pend1:
                        f()
                    pend1.clear()
                    if not do1:
                        return
                    P1 = xpend1.pop(t1)
                    Q1 = hphase(1, t1, P1)
                    hn1 = gates(1, t1, prev1, P1, Q1)

                def tail0():
                    nonlocal prev0
                    if t0 + 1 <= T and t0 + 1 not in xpend0:
                        xpend0[t0 + 1] = xphase(0, t0 + 1)
                    if not do0:
                        return
                    TT0 = transposes(0, t0, hn0)
                    copies(0, t0, TT0, hn0)
                    replicas_tt(0, t0, TT0)
                    if ex0:
                        exchange_comm(0, t0)()
                        pend0.append(
                            lambda t=t0, p=hn0: (exchange_patch(0, t, p),
                                                 replicas_sb(0, t)))
                    prev0 = hn0

                def tail1():
                    nonlocal prev1
                    if 1 <= t1 + 1 <= T and t1 + 1 not in xpend1:
                        xpend1[t1 + 1] = xphase(1, t1 + 1)
                    if not do1:
                        return
                    TT1 = transposes(1, t1, hn1)
                    copies(1, t1, TT1, hn1)
                    replicas_tt(1, t1, TT1)
                    if ex1:
                        exchange_comm(1, t1)()
                        pend1.append(
                            lambda t=t1, p=hn1: (exchange_patch(1, t, p),
                                                 replicas_sb(1, t)))
                    prev1 = hn1

                def xahead():
                    # pre-emit both x lookaheads so the PE has fill while a
                    # pending halo patch lands
                    if t0 + 1 <= T and t0 + 1 not in xpend0:
                        xpend0[t0 + 1] = xphase(0, t0 + 1)
                    if 1 <= t1 + 1 <= T and t1 + 1 not in xpend1:
                        xpend1[t1 + 1] = xphase(1, t1 + 1)

                # x lookaheads are emitted BEFORE the transpose/copy blocks:
                # their tap matmuls read older ys0a/ys5 slots, and emitting
                # them after this slot's copies serializes them behind those
                # writes (PE gap -> HAM clock drop). After a layer-0
                # exchange, additionally lead with layer 1 so the PE has
                # collective-independent work while the patch lands.
                if swap:
                    head1(); xahead(); head0(); tail1(); tail0()
                else:
                    head0(); head1(); xahead(); tail0(); tail1()
                swap = ex0

    nc.compile()
    return nc


def prep_inputs(xs, W_i0, b_i0, W_h0, W_i1, b_i1, W_h1, T):
    """Host-side sharding/packing -> per-core in_maps."""
    xs = np.asarray(xs, np.float32)
    pads = 34
    xs_p = np.zeros((L + 2 * pads + 4, T, C), np.float32)
    xs_p[pads:pads + L] = xs[:, :T]

    def pack_w(Wi, bi, Wh):
        Wi = np.asarray(Wi, np.float32)
        Wh = np.asarray(Wh, np.float32)
        wim = np.ascontiguousarray(Wi.transpose(1, 0, 2)[:128]).reshape(128, KW * G3)
        whm = np.ascontiguousarray(Wh.transpose(1, 0, 2)[:128]).reshape(128, KW * G3)
        wilA = np.zeros((A5, G3), np.float32)
        whlA = np.zeros((A5, G3), np.float32)
        for k in range(4):
            wilA[k * 32: k * 32 + CL] = Wi[k, 128:C, :]
            whlA[k * 32: k * 32 + CL] = Wh[k, 128:C, :]
        # layer-0 x A-block pairs with host-packed xsba (tap-4 in the gap
        # rows, ones row at 88); layer-1 x A-block pairs with the on-chip
        # ys5 (zero gaps, ones row at 118); tap-4 of the on-chip state goes
        # through the separate B matmuls instead.
        wil0 = wilA.copy()
        for r, ch in _GAP_PACK:
            wil0[r] = Wi[4, 128 + ch, :]
        wil0[_ONES_ROW] = np.asarray(bi, np.float32)
        wil1 = wilA.copy()
        wil1[A5 - 1] = np.asarray(bi, np.float32)
        whlB = np.ascontiguousarray(Wh[4, 128:C, :])
        wilB = np.ascontiguousarray(Wi[4, 128:C, :])
        cv = lambda a: a.astype(BF16)
        return cv(wim), cv(whm), cv(wil0), cv(wil1), cv(whlA), cv(whlB), \
            cv(wilB)

    packed = [pack_w(W_i0, b_i0, W_h0), pack_w(W_i1, b_i1, W_h1)]
    wmm = [np.concatenate([p[0], p[1]], axis=1) for p in packed]
    wA = [np.concatenate([packed[0][2], packed[0][4]], axis=1),
          np.concatenate([packed[1][3], packed[1][4]], axis=1)]
    wB = np.concatenate([packed[0][5], packed[1][5], packed[1][6]], axis=1)
    identb = np.eye(128, dtype=np.float32).astype(BF16)

    in_maps = []
    for i in range(NCORES):
        blk = xs_p[OWN * i: OWN * i + SW]          # (134, T, C)
        blkT = np.ascontiguousarray(blk.transpose(2, 1, 0))  # (C, T, 134)
        xsa = blkT[:128].reshape(128, T * SW)
        xsba = np.zeros((AR, T, SW), np.float32)
        for k in range(4):
            xsba[k * 32: k * 32 + CL, :, 0:128] = \
                blk[k:k + 128, :, 128:C].transpose(2, 1, 0)
        pos = np.arange(128) + OWN * i - 32
        validv = ((pos >= 0) & (pos < L)).astype(np.float32)
        tap4 = blk[4:4 + 128, :, 128:C].transpose(2, 1, 0)  # (CL, T, 128)
        for r, ch in _GAP_PACK:
            if ch < CL:
                xsba[r, :, 0:128] = tap4[ch]
        xsba[_ONES_ROW, :, 0:128] = validv[None, :]

        mask = np.zeros((8, 64), np.float32)
        if i + 1 < NCORES:
            mask[i + 1, 0:32] = 1.0
        if i - 1 >= 0:
            mask[i - 1, 32:64] = 1.0
        maska = np.tile(mask.reshape(1, 512), (128, 1))

        im = {
            "xsT_a": xsa.astype(BF16),
            "xsT_bA": np.ascontiguousarray(xsba.reshape(AR, T * SW)).astype(BF16),
            "mask_a": np.ascontiguousarray(maska).astype(BF16),
            "mask_b": np.ascontiguousarray(maska[:CL]).astype(BF16),
            "valid128": validv.reshape(1, 128).astype(BF16),
            "valid25": np.tile(validv, W0).reshape(1, W0 * 128).astype(BF16),
            "validp": np.ascontiguousarray(validv.reshape(128, 1)),
            "identb": identb,
        }
        for l in range(2):
            im[f"wmm{l}"] = wmm[l]
            im[f"wA{l}"] = wA[l]
        im["wB"] = wB
        in_maps.append(im)
    return in_maps


_BUILD_CACHE = {}


def run(inputs, T=96, trace=False):
    global LAST_EXEC_NS
    from concourse import bass_utils
    if T not in _BUILD_CACHE:
        _BUILD_CACHE[T] = build(T)
    nc = _BUILD_CACHE[T]
    in_maps = prep_inputs(T=T, **inputs)
    if trace:
        _install_ntff_hook()
    res = bass_utils.run_bass_kernel_spmd(
        nc, in_maps, core_ids=list(range(NCORES)), trace=trace)
    LAST_EXEC_NS = res.exec_time_ns
    ys = np.empty((L, T, H), np.float32)
    for i in range(NCORES):
        ys[OWN * i: OWN * (i + 1)] = \
            res.results[i]["out"].astype(np.float32).transpose(1, 0, 2)
    return ys


def kernel(**inputs):
    trace = bool(int(os.environ.get("BASS_KERNEL_TRACE", "0")))
    return run(inputs, T=96, trace=trace)


# revision 36
# speedup vs baseline: 1.0197x; 1.0197x over previous
"""Trainium2 Bass kernel for a 2-layer ConvGRU (L=512, T=96, C=H=150, K=5).

Sharding: spatial axis L split over 8 NeuronCores (64 owned positions each).
Each core computes a 128-wide region (owned + 32 halo per side). Halo
validity decays 2 positions/step (kernel-5 SAME conv); it is refreshed every
16 steps by a ReduceScatter halo exchange (per-core one-hot masks route each
core's boundary strips to its neighbours' slots, keeping the SPMD program
uniform). Layer-0 exchanges sit at t=16k, layer-1 at t=16k+8.

v2 restructure vs the original baseline (959us):
- h state kept in bf16 end-to-end; PE transposes cost 1 cyc/row.
- gate math rewritten as h' = s*n + (prev - s*prev) with s = sigmoid(-zpre)
  computed directly on ACT (scale=-1), so only 2 DVE ops follow the tanh.
- replica copies moved from DVE to the otherwise idle GpSimd engine.
- x-side matmuls of step t+1 are emitted between this slot's h matmuls and
  the gate-dependent PE transposes, so the PE never idles during gate math
  (keeps the HAM activity monitor at the full 2.4 GHz clock).
- exchange rs DMAs moved to the SP queue so the Pool replica stream never
  blocks on the collective.

All input reshaping (channel-major transposes, tap-shifted im2col packing,
bias/valid rows, routing masks) is done host-side in numpy inside kernel().
"""

import os
import sys
import types

import numpy as np
import ml_dtypes

BF16 = ml_dtypes.bfloat16

if "/opt/trn_rl_repo" not in sys.path:
    sys.path.insert(0, "/opt/trn_rl_repo")


def _install_ntff_hook():
    # antenv.axon_hooks is absent from this image; recreate the registry and
    # register the ctypes NTFF hook so trace=True yields exec_time_ns.
    try:
        import antenv
        if "antenv.axon_hooks" in sys.modules:
            return True
        mod = types.ModuleType("antenv.axon_hooks")
        _hook = [None]
        mod.set_axon_ntff_profile_hook = lambda h: _hook.__setitem__(0, h)
        mod.get_axon_ntff_profile_hook = lambda: _hook[0]
        sys.modules["antenv.axon_hooks"] = mod
        antenv.axon_hooks = mod
        from trn_agent_boot.trn_boot import _ntff_profile_via_ctypes
        mod.set_axon_ntff_profile_hook(
            _ntff_profile_via_ctypes("/opt/axon/libaxon_pjrt.so"))
        return True
    except Exception:
        return False


L, C, H, KW = 512, 150, 150, 5
G3 = 3 * H            # 450
NCORES = 8
OWN = 64              # owned positions per core
SW = 134              # stored width per timestep (128 region + pads)
W0 = 25               # rolling slots for layer-0 output trajectory
LAG = 4               # emission lag of layer 1 behind layer 0
CL = 22               # channel remainder (150 - 128)
AR = 118              # 4 tap blocks at partitions {0,32,64,96}, 22 rows each
A5 = 119              # A-block rows: AR gap-filled + ones/bias row at 118

# tap-4 channel -> spare row inside the 118-row aligned block (layer-0 xs
# host packing only): gaps 22..31, 54..63, 86..87; ones/bias row at 88.
_GAP_PACK = [(22 + c, c) for c in range(10)] + \
    [(54 + c - 10, c) for c in range(10, 20)] + \
    [(86 + c - 20, c) for c in range(20, 22)]
_ONES_ROW = 88        # ones/bias row for the host-packed layer-0 x A-block

LAST_EXEC_NS = None


def build(T):
    import concourse.bacc as bacc
    import concourse.mybir as mybir
    import concourse.tile as tile

    f32 = mybir.dt.float32
    bf16 = mybir.dt.bfloat16
    AF = mybir.ActivationFunctionType
    nc = bacc.Bacc("TRN2", target_bir_lowering=False, debug=False,
                   num_devices=NCORES)

    d_xsa = nc.dram_tensor("xsT_a", [128, T * SW], bf16, kind="ExternalInput")
    d_xsba = nc.dram_tensor("xsT_bA", [AR, T * SW], bf16, kind="ExternalInput")
    # weights are packed host-side into few wide tensors: DMA issue cost is
    # per partition-row descriptor, so fewer/wider transfers start faster
    wnames = [("wmm0", [128, 2 * KW * G3]), ("wmm1", [128, 2 * KW * G3]),
              ("wA0", [A5, 2 * G3]), ("wA1", [A5, 2 * G3]),
              ("wB", [CL, 3 * G3])]
    d_w = {nm: nc.dram_tensor(nm, sh, bf16, kind="ExternalInput")
           for nm, sh in wnames}
    d_maska = nc.dram_tensor("mask_a", [128, 512], bf16, kind="ExternalInput")
    d_maskb = nc.dram_tensor("mask_b", [CL, 512], bf16, kind="ExternalInput")
    d_valid = nc.dram_tensor("valid128", [1, 128], bf16, kind="ExternalInput")
    d_valid25 = nc.dram_tensor("valid25", [1, W0 * 128], bf16,
                               kind="ExternalInput")
    d_validp = nc.dram_tensor("validp", [128, 1], f32, kind="ExternalInput")
    d_identb = nc.dram_tensor("identb", [128, 128], bf16, kind="ExternalInput")
    d_out = nc.dram_tensor("out", [T, OWN, H], bf16, kind="ExternalOutput")

    with tile.TileContext(nc) as tc:
        with tc.tile_pool(name="persist", bufs=1) as pp, \
             tc.tile_pool(name="work", bufs=2) as wp, \
             tc.tile_pool(name="hnew", bufs=3) as hp, \
             tc.tile_pool(name="psP", bufs=2, space="PSUM") as psP, \
             tc.tile_pool(name="psQ", bufs=2, space="PSUM") as psQ, \
             tc.tile_pool(name="dram", bufs=2, space="DRAM") as dp:

            # ---- persistent SBUF tiles ----
            xsa = pp.tile([128, T * SW], bf16, tag="xsa", name="xsa")
            xsba = pp.tile([AR, T * SW], bf16, tag="xsba", name="xsba")
            w_sb = {nm: pp.tile(sh, bf16, tag=nm, name=nm) for nm, sh in wnames}
            maska = pp.tile([128, 512], bf16, tag="maska", name="maska")
            maskb = pp.tile([CL, 512], bf16, tag="maskb", name="maskb")
            valid = pp.tile([1, 128], bf16, tag="valid", name="valid")
            validp = pp.tile([128, 1], f32, tag="validp", name="validp")
            identb = pp.tile([128, 128], bf16, tag="identb", name="identb")
            hz0 = pp.tile([128, 152], bf16, tag="hz0", name="hz0")
            ys0a = pp.tile([128, W0 * SW], bf16, tag="ys0a", name="ys0a")
            ys0b = pp.tile([CL, W0 * SW], bf16, tag="ys0b", name="ys0b")
            ys5 = pp.tile([A5, W0 * 128], bf16, tag="ys5", name="ys5")
            h1a = pp.tile([128, SW], bf16, tag="h1a", name="h1a")
            h1b = pp.tile([CL, SW], bf16, tag="h1b", name="h1b")
            h51 = pp.tile([A5, 128], bf16, tag="h51", name="h51")

            # ---- init: loads + zero state ----
            # DMA issue cost is ~29ns/descriptor (one per partition row), so
            # order matters more than size: step-1 needs land first, big
            # trailing chunks later; xsa/weights ride the sync queue while
            # xsba rides the scalar HWDGE queue in parallel.
            tcuts = [0, 4, 16, 48, 96]
            ci = 0
            for a, b in zip(tcuts, tcuts[1:]):
                a, b = a * SW, min(b, T) * SW
                if b > a:
                    nc.sync.dma_start(xsa[:, a:b], d_xsa[:, a:b])
                    nc.scalar.dma_start(xsba[:, a:b], d_xsba[:, a:b])
                if ci == 0:
                    nc.sync.dma_start(w_sb["wmm0"][:], d_w["wmm0"][:])
                    nc.sync.dma_start(w_sb["wA0"][:], d_w["wA0"][:])
                    nc.scalar.dma_start(w_sb["wB"][:], d_w["wB"][:])
                    nc.scalar.dma_start(validp[:], d_validp[:])
                    nc.scalar.dma_start(identb[:], d_identb[:])
                    nc.scalar.dma_start(ys5[A5 - 1:A5, :], d_valid25[0:1, :])
                elif ci == 1:
                    nc.sync.dma_start(w_sb["wmm1"][:], d_w["wmm1"][:])
                    nc.scalar.dma_start(w_sb["wA1"][:], d_w["wA1"][:])
                ci += 1
            nc.scalar.dma_start(maska[:], d_maska[:])
            nc.scalar.dma_start(maskb[:], d_maskb[:])
            nc.scalar.dma_start(valid[:], d_valid[:])
            wrm_in = dp.tile([8, 64], f32, tag="wrm_in", name="wrm_in")
            wrm_out = dp.tile([1, 64], f32, tag="wrm_out", name="wrm_out")
            wrm_sb = wp.tile([8, 64], f32, tag="wrm_sb", name="wrm_sb")
            nc.vector.memset(wrm_sb[:], 0.0)
            nc.sync.dma_start(wrm_in[:], wrm_sb[:])
            nc.gpsimd.collective_compute(
                "ReduceScatter", mybir.AluOpType.add,
                replica_groups=[list(range(NCORES))],
                ins=[wrm_in[:].opt()], outs=[wrm_out[:].opt()])
            nc.vector.memset(ys0a[:], 0.0)
            nc.vector.memset(ys0b[:], 0.0)
            nc.vector.memset(hz0[:], 0.0)
            nc.gpsimd.memset(ys5[0:A5 - 1, :], 0.0)
            nc.gpsimd.memset(h1a[:], 0.0)
            nc.gpsimd.memset(h1b[:], 0.0)
            nc.gpsimd.memset(h51[:], 0.0)

            def xphase(l, t):
                """x-side gate pre-activations of step t into a fresh PSUM
                bank (450 wide, bias via the A-block ones row)."""
                P = psP.tile([128, G3], f32, tag=f"P{l}", name=f"P{l}")
                if l == 0:
                    xa, xbase = xsa, (t - 1) * SW
                    xA, xAb, xAr = xsba, (t - 1) * SW, AR
                else:
                    xa, xbase = ys0a, (t % W0) * SW
                    xA, xAb, xAr = ys5, (t % W0) * 128, A5
                for k in range(KW):
                    nc.tensor.matmul(
                        P[:, 0:G3], xa[:, xbase + k: xbase + k + 128],
                        w_sb[f"wmm{l}"][:, k * G3:(k + 1) * G3],
                        start=(k == 0), stop=False)
                if l == 1:
                    nc.tensor.matmul(
                        P[:, 0:G3], ys0b[0:CL, (t % W0) * SW + 4:
                                         (t % W0) * SW + 132],
                        w_sb["wB"][0:CL, 2 * G3:3 * G3],
                        start=False, stop=False)
                nc.tensor.matmul(P[:, 0:G3], xA[0:xAr, xAb: xAb + 128],
                                 w_sb[f"wA{l}"][0:xAr, 0:G3],
                                 start=False, stop=False)
                return P

            def hphase(l, t, P):
                """h-side conv accumulation: [r|z] into P, n into Q."""
                Q = psQ.tile([128, 512], f32, tag=f"Q{l}", name=f"Q{l}")
                if l == 0:
                    ha, hbase = ys0a, ((t - 1) % W0) * SW
                    hA, hAb = ys5, ((t - 1) % W0) * 128
                    hb, hBb = ys0b, ((t - 1) % W0) * SW + 4
                else:
                    ha, hbase = h1a, 0
                    hA, hAb = h51, 0
                    hb, hBb = h1b, 4
                hm = KW * G3
                bb = G3 * l
                for k in range(KW):
                    nc.tensor.matmul(
                        P[:, 0:2 * H], ha[:, hbase + k: hbase + k + 128],
                        w_sb[f"wmm{l}"][:, hm + k * G3: hm + k * G3 + 2 * H],
                        start=False, stop=False)
                    nc.tensor.matmul(
                        Q[:, 0:H], ha[:, hbase + k: hbase + k + 128],
                        w_sb[f"wmm{l}"][:, hm + k * G3 + 2 * H:
                                        hm + (k + 1) * G3],
                        start=(k == 0), stop=False)
                nc.tensor.matmul(P[:, 0:2 * H], hb[0:CL, hBb: hBb + 128],
                                 w_sb["wB"][0:CL, bb: bb + 2 * H],
                                 start=False, stop=False)
                nc.tensor.matmul(Q[:, 0:H], hb[0:CL, hBb: hBb + 128],
                                 w_sb["wB"][0:CL, bb + 2 * H: bb + G3],
                                 start=False, stop=False)
                nc.tensor.matmul(P[:, 0:2 * H], hA[0:A5, hAb: hAb + 128],
                                 w_sb[f"wA{l}"][0:A5, G3: G3 + 2 * H],
                                 start=False, stop=True)
                nc.tensor.matmul(Q[:, 0:H], hA[0:A5, hAb: hAb + 128],
                                 w_sb[f"wA{l}"][0:A5, G3 + 2 * H: 2 * G3],
                                 start=False, stop=True)
                return Q

            def gates(l, t, prev, P, Q):
                """h' = n + z*(prev - n); one 300-wide sigmoid frees the P
                bank as early as possible (the next x-phase's PSUM anti-dep
                waits on it)."""
                rz = wp.tile([128, 304], f32, tag=f"rz{l}", name=f"rz{l}")
                nc.scalar.activation(rz[:, 0:2 * H], P[:, 0:2 * H], AF.Sigmoid)
                ta = wp.tile([128, 152], f32, tag=f"ta{l}", name=f"ta{l}")
                nc.vector.tensor_mul(ta[:, 0:H], Q[:, 0:H], rz[:, 0:H])
                tb = wp.tile([128, 152], f32, tag=f"tb{l}", name=f"tb{l}")
                nc.vector.tensor_add(tb[:, 0:H], ta[:, 0:H], P[:, 2 * H:G3])
                # scale by the valid-position indicator: pins n to exactly 0
                # outside the global [0,L) range (reference SAME padding)
                tn = wp.tile([128, 152], f32, tag=f"tn{l}", name=f"tn{l}")
                nc.scalar.activation(tn[:, 0:H], tb[:, 0:H], AF.Tanh,
                                     scale=validp[:, 0:1])
                tc = wp.tile([128, 152], f32, tag=f"tc{l}", name=f"tc{l}")
                nc.vector.tensor_sub(tc[:, 0:H], prev[:, 0:H], tn[:, 0:H])
                td = wp.tile([128, 152], f32, tag=f"td{l}", name=f"td{l}")
                nc.vector.tensor_mul(td[:, 0:H], rz[:, H:2 * H], tc[:, 0:H])
                hn = hp.tile([128, 152], bf16, tag=f"hn{l}", name=f"hn{l}")
                nc.vector.tensor_add(hn[:, 0:H], tn[:, 0:H], td[:, 0:H])
                return hn

            def transposes(l, t, hn):
                """h_new back to channel-major via 2 PE transposes (bf16)."""
                TT = psQ.tile([128, 256], bf16, tag=f"Q{l}", name=f"TT{l}")
                nc.tensor.matmul(TT[:, 0:128], hn[:, 0:128], identb[:],
                                 is_transpose=True, start=True, stop=True)
                nc.tensor.matmul(TT[0:CL, 128:256], hn[:, 128:H], identb[:],
                                 is_transpose=True, start=True, stop=True)
                return TT

            def copies(l, t, TT, hn):
                if l == 0:
                    st = (t % W0) * SW
                    nc.vector.tensor_copy(ys0a[:, st + 2: st + 130],
                                          TT[:, 0:128])
                    nc.scalar.copy(ys0b[0:CL, st + 2: st + 130],
                                   TT[0:CL, 128:256])
                else:
                    nc.vector.tensor_copy(h1a[:, 2:130], TT[:, 0:128])
                    nc.scalar.copy(h1b[0:CL, 2:130], TT[0:CL, 128:256])
                    nc.sync.dma_start(d_out[t - 1, :, :], hn[32:96, 0:H])

            def replicas_tt(l, t, TT):
                """Rebuild the 32-aligned tap blocks 0..3 straight from the
                transpose PSUM (no wait on the ysb copy). Strip col j lives
                at TT col 126+j; the out-of-strip edge columns only feed
                halo-edge output positions, which the decay budget already
                writes off. k=3 is clipped one column to avoid reading
                stale PSUM beyond the transpose (its last A column keeps
                old slot data — also halo-edge-only)."""
                if l == 0:
                    s5 = (t % W0) * 128
                    A_ = ys5
                else:
                    s5 = 0
                    A_ = h51
                for k in range(4):
                    w = 127 if k == 3 else 128
                    eng = nc.vector if k < 2 else nc.scalar
                    src = TT[0:CL, 126 + k: 126 + k + w]
                    if eng is nc.vector:
                        eng.tensor_copy(A_[k * 32: k * 32 + CL, s5: s5 + w], src)
                    else:
                        eng.copy(A_[k * 32: k * 32 + CL, s5: s5 + w], src)

            def replicas_sb(l, t):
                """Post-exchange rebuild of the tap blocks from the patched
                channel-remainder strip (proper zero pads at the edges)."""
                if l == 0:
                    st = (t % W0) * SW
                    s5 = (t % W0) * 128
                    A_, B_ = ys5, ys0b
                else:
                    st, s5 = 0, 0
                    A_, B_ = h51, h1b
                for k in range(4):
                    src = B_[0:CL, st + k: st + k + 128]
                    if k < 2:
                        nc.vector.tensor_copy(
                            A_[k * 32: k * 32 + CL, s5: s5 + 128], src)
                    else:
                        nc.scalar.copy(
                            A_[k * 32: k * 32 + CL, s5: s5 + 128], src)

            def exchange_comm(l, t):
                """ReduceScatter halo refresh of layer l's state at step t.
                Mask muls on DVE, rs DMAs on SP; only the CC trigger sits on
                the Pool stream (emitted after this slot's replicas)."""
                if l == 0:
                    A, B, base = ys0a, ys0b, (t % W0) * SW
                else:
                    A, B, base = h1a, h1b, 0
                rsa = wp.tile([128, 512], bf16, tag="rsa", name="rsa")
                rsb = wp.tile([CL, 512], bf16, tag="rsb", name="rsb")
                # q in [0,32): receivers' left halo <- my owned last 32
                # q in [32,64): receivers' right halo <- my owned first 32
                for dq, sc in ((0, 66), (32, 34)):
                    nc.vector.tensor_mul(
                        rsa[:].rearrange("c (s q) -> c s q", s=8)[:, :, dq:dq + 32],
                        A[:, base + sc: base + sc + 32]
                        .unsqueeze(1).broadcast_to([128, 8, 32]),
                        maska[:].rearrange("c (s q) -> c s q", s=8)[:, :, dq:dq + 32])
                    nc.vector.tensor_mul(
                        rsb[0:CL].rearrange("c (s q) -> c s q", s=8)[:, :, dq:dq + 32],
                        B[0:CL, base + sc: base + sc + 32]
                        .unsqueeze(1).broadcast_to([CL, 8, 32]),
                        maskb[:].rearrange("c (s q) -> c s q", s=8)[:, :, dq:dq + 32])
                rs_in = dp.tile([8, C, 64], bf16, tag="rs_in", name="rs_in")
                rs_out = dp.tile([C, 64], bf16, tag="rs_out", name="rs_out")
                nc.sync.dma_start(rs_in[:, 0:128, :].transpose([1, 0, 2]),
                                  rsa[:].rearrange("c (s q) -> c s q", s=8))
                nc.sync.dma_start(rs_in[:, 128:C, :].transpose([1, 0, 2]),
                                  rsb[0:CL].rearrange("c (s q) -> c s q", s=8))

                def go():
                    nc.gpsimd.collective_compute(
                        "ReduceScatter", mybir.AluOpType.add,
                        replica_groups=[list(range(NCORES))],
                        ins=[rs_in[:].opt()], outs=[rs_out[:].opt()])
                    nc.sync.dma_start(A[:, base + 2: base + 34],
                                      rs_out[0:128, 0:32])
                    nc.sync.dma_start(A[:, base + 98: base + 130],
                                      rs_out[0:128, 32:64])
                    nc.sync.dma_start(B[0:CL, base + 2: base + 34],
                                      rs_out[128:C, 0:32])
                    nc.sync.dma_start(B[0:CL, base + 98: base + 130],
                                      rs_out[128:C, 32:64])
                return go

            def exchange_patch(l, t, prev):
                """Deferred: re-materialise patched halo rows of the
                position-major h_t copy via PE transposes (emitted next slot
                so in-order engine streams don't stall on the collective)."""
                if l == 0:
                    A, B, base = ys0a, ys0b, (t % W0) * SW
                else:
                    A, B, base = h1a, h1b, 0
                TX = psQ.tile([32, 512], bf16, tag=f"Q{l}", name=f"TX{l}")
                nc.tensor.matmul(TX[0:32, 0:128], A[:, base + 2: base + 34],
                                 identb[:], is_transpose=True,
                                 start=True, stop=True)
                nc.tensor.matmul(TX[0:32, 128:256], A[:, base + 98: base + 130],
                                 identb[:], is_transpose=True,
                                 start=True, stop=True)
                nc.tensor.matmul(TX[0:32, 256:256 + CL],
                                 B[0:CL, base + 2: base + 34],
                                 identb[0:CL, 0:CL], is_transpose=True,
                                 start=True, stop=True)
                nc.tensor.matmul(TX[0:32, 288:288 + CL],
                                 B[0:CL, base + 98: base + 130],
                                 identb[0:CL, 0:CL], is_transpose=True,
                                 start=True, stop=True)
                nc.vector.tensor_copy(prev[0:32, 0:128], TX[0:32, 0:128])
                nc.vector.tensor_copy(prev[96:128, 0:128], TX[0:32, 128:256])
                nc.vector.tensor_copy(prev[0:32, 128:H], TX[0:32, 256:256 + CL])
                nc.vector.tensor_copy(prev[96:128, 128:H],
                                      TX[0:32, 288:288 + CL])

            # ---- main pipelined emission ----
            prev0, prev1 = hz0, hz0
            hn0 = hn1 = None
            pend0, pend1 = [], []
            xpend0, xpend1 = {}, {}
            xpend0[1] = xphase(0, 1)
            swap = False

            for t0 in range(1, T + LAG + 1):
                t1 = t0 - LAG
                do0 = t0 <= T
                do1 = 1 <= t1 <= T
                ex0 = do0 and t0 % 16 == 0
                ex1 = do1 and t1 % 16 == 8 and t1 < T

                def head0():
                    nonlocal hn0
                    for f in pend0:
                        f()
                    pend0.clear()
                    if not do0:
                        return
                    P0 = xpend0.pop(t0)
                    Q0 = hphase(0, t0, P0)
                    hn0 = gates(0, t0, prev0, P0, Q0)

                def head1():
                    nonlocal hn1
                    for f in pend1:
                        f()
                    pend1.clear()
                    if not do1:
                        return
                    P1 = xpend1.pop(t1)
                    Q1 = hphase(1, t1, P1)
                    hn1 = gates(1, t1, prev1, P1, Q1)

                def tail0():
                    nonlocal prev0
                    if t0 + 1 <= T and t0 + 1 not in xpend0:
                        xpend0[t0 + 1] = xphase(0, t0 + 1)
                    if not do0:
                        return
                    TT0 = transposes(0, t0, hn0)
                    copies(0, t0, TT0, hn0)
                    replicas_tt(0, t0, TT0)
                    if ex0:
                        exchange_comm(0, t0)()
                        pend0.append(
                            lambda t=t0, p=hn0: (exchange_patch(0, t, p),
                                                 replicas_sb(0, t)))
                    prev0 = hn0

                def tail1():
                    nonlocal prev1
                    if 1 <= t1 + 1 <= T and t1 + 1 not in xpend1:
                        xpend1[t1 + 1] = xphase(1, t1 + 1)
                    if not do1:
                        return
                    TT1 = transposes(1, t1, hn1)
                    copies(1, t1, TT1, hn1)
                    replicas_tt(1, t1, TT1)
                    if ex1:
                        exchange_comm(1, t1)()
                        pend1.append(
                            lambda t=t1, p=hn1: (exchange_patch(1, t, p),
                                                 replicas_sb(1, t)))
                    prev1 = hn1

                def xahead():
                    # pre-emit both x lookaheads so the PE has fill while a
                    # pending halo patch lands
                    if t0 + 1 <= T and t0 + 1 not in xpend0:
                        xpend0[t0 + 1] = xphase(0, t0 + 1)
                    if 1 <= t1 + 1 <= T and t1 + 1 not in xpend1:
                        xpend1[t1 + 1] = xphase(1, t1 + 1)

                # x lookaheads are emitted BEFORE the transpose/copy blocks:
                # their tap matmuls read older ys0a/ys5 slots, and emitting
                # them after this slot's copies serializes them behind those
                # writes (PE gap -> HAM clock drop). After a layer-0
                # exchange, additionally lead with layer 1 so the PE has
                # collective-independent work while the patch lands.
                if swap:
                    head1(); xahead(); head0(); tail1(); tail0()
                else:
                    head0(); head1(); xahead(); tail0(); tail1()
                swap = ex0

    nc.compile()
    return nc


def prep_inputs(xs, W_i0, b_i0, W_h0, W_i1, b_i1, W_h1, T):
    """Host-side sharding/packing -> per-core in_maps."""
    xs = np.asarray(xs, np.float32)
    pads = 34
    xs_p = np.zeros((L + 2 * pads + 4, T, C), np.float32)
    xs_p[pads:pads + L] = xs[:, :T]

    def pack_w(Wi, bi, Wh):
        Wi = np.asarray(Wi, np.float32)
        Wh = np.asarray(Wh, np.float32)
        wim = np.ascontiguousarray(Wi.transpose(1, 0, 2)[:128]).reshape(128, KW * G3)
        whm = np.ascontiguousarray(Wh.transpose(1, 0, 2)[:128]).reshape(128, KW * G3)
        wilA = np.zeros((A5, G3), np.float32)
        whlA = np.zeros((A5, G3), np.float32)
        for k in range(4):
            wilA[k * 32: k * 32 + CL] = Wi[k, 128:C, :]
            whlA[k * 32: k * 32 + CL] = Wh[k, 128:C, :]
        # layer-0 x A-block pairs with host-packed xsba (tap-4 in the gap
        # rows, ones row at 88); layer-1 x A-block pairs with the on-chip
        # ys5 (zero gaps, ones row at 118); tap-4 of the on-chip state goes
        # through the separate B matmuls instead.
        wil0 = wilA.copy()
        for r, ch in _GAP_PACK:
            wil0[r] = Wi[4, 128 + ch, :]
        wil0[_ONES_ROW] = np.asarray(bi, np.float32)
        wil1 = wilA.copy()
        wil1[A5 - 1] = np.asarray(bi, np.float32)
        whlB = np.ascontiguousarray(Wh[4, 128:C, :])
        wilB = np.ascontiguousarray(Wi[4, 128:C, :])
        cv = lambda a: a.astype(BF16)
        return cv(wim), cv(whm), cv(wil0), cv(wil1), cv(whlA), cv(whlB), \
            cv(wilB)

    packed = [pack_w(W_i0, b_i0, W_h0), pack_w(W_i1, b_i1, W_h1)]
    wmm = [np.concatenate([p[0], p[1]], axis=1) for p in packed]
    wA = [np.concatenate([packed[0][2], packed[0][4]], axis=1),
          np.concatenate([packed[1][3], packed[1][4]], axis=1)]
    wB = np.concatenate([packed[0][5], packed[1][5], packed[1][6]], axis=1)
    identb = np.eye(128, dtype=np.float32).astype(BF16)

    in_maps = []
    for i in range(NCORES):
        blk = xs_p[OWN * i: OWN * i + SW]          # (134, T, C)
        blkT = np.ascontiguousarray(blk.transpose(2, 1, 0))  # (C, T, 134)
        xsa = blkT[:128].reshape(128, T * SW)
        xsba = np.zeros((AR, T, SW), np.float32)
        for k in range(4):
            xsba[k * 32: k * 32 + CL, :, 0:128] = \
                blk[k:k + 128, :, 128:C].transpose(2, 1, 0)
        pos = np.arange(128) + OWN * i - 32
        validv = ((pos >= 0) & (pos < L)).astype(np.float32)
        tap4 = blk[4:4 + 128, :, 128:C].transpose(2, 1, 0)  # (CL, T, 128)
        for r, ch in _GAP_PACK:
            if ch < CL:
                xsba[r, :, 0:128] = tap4[ch]
        xsba[_ONES_ROW, :, 0:128] = validv[None, :]

        mask = np.zeros((8, 64), np.float32)
        if i + 1 < NCORES:
            mask[i + 1, 0:32] = 1.0
        if i - 1 >= 0:
            mask[i - 1, 32:64] = 1.0
        maska = np.tile(mask.reshape(1, 512), (128, 1))

        im = {
            "xsT_a": xsa.astype(BF16),
            "xsT_bA": np.ascontiguousarray(xsba.reshape(AR, T * SW)).astype(BF16),
            "mask_a": np.ascontiguousarray(maska).astype(BF16),
            "mask_b": np.ascontiguousarray(maska[:CL]).astype(BF16),
            "valid128": validv.reshape(1, 128).astype(BF16),
            "valid25": np.tile(validv, W0).reshape(1, W0 * 128).astype(BF16),
            "validp": np.ascontiguousarray(validv.reshape(128, 1)),
            "identb": identb,
        }
        for l in range(2):
            im[f"wmm{l}"] = wmm[l]
            im[f"wA{l}"] = wA[l]
        im["wB"] = wB
        in_maps.append(im)
    return in_maps


_BUILD_CACHE = {}


def run(inputs, T=96, trace=False):
    global LAST_EXEC_NS
    from concourse import bass_utils
    if T not in _BUILD_CACHE:
        _BUILD_CACHE[T] = build(T)
    nc = _BUILD_CACHE[T]
    in_maps = prep_inputs(T=T, **inputs)
    if trace:
        _install_ntff_hook()
    res = bass_utils.run_bass_kernel_spmd(
        nc, in_maps, core_ids=list(range(NCORES)), trace=trace)
    LAST_EXEC_NS = res.exec_time_ns
    ys = np.empty((L, T, H), np.float32)
    for i in range(NCORES):
        ys[OWN * i: OWN * (i + 1)] = \
            res.results[i]["out"].astype(np.float32).transpose(1, 0, 2)
    return ys


def kernel(**inputs):
    trace = bool(int(os.environ.get("BASS_KERNEL_TRACE", "0")))
    return run(inputs, T=96, trace=trace)
